# revision 81
# baseline (speedup 1.0000x reference)
"""Trainium2 Bass kernel for CausalWaveletFieldAttention.

Full-input contract: kernel(**inputs) takes the complete (unsharded) numpy
inputs and returns the full [8, 2048, 1024] float32 output.

Sharding: pure data-parallel over batch B=8 -> one batch element per
NeuronCore (8 cores), zero collectives (the head-coupling einsum mixes heads
within a batch element only).

Per-core pipeline (x pre-transposed to feature-major on host, bf16 compute,
fp32 PSUM accumulation, fp8 DoubleRow for the k and gate projections):
  1. k = x @ Wk.T       (TensorE, fp8 DoubleRow; contraction index
     c = 256*ic + 128*j + p so the fp8 operands are plain row-slices of the
     feature-major [D, N] fp8 copy of x -- no host interleave needed)
  2. k2 = Square(k + bk) (ScalarE, fp8 output), per-head sums of 64
     partitions via an fp8 DoubleRow selector matmul (TensorE) ->
     kmag = sqrt(.) (ScalarE)
  3. v = x @ Wv.T with v output channels permuted to d-major order
     (channel c~ = d*16 + h); field = (v + bv) * kmag (fused DVE op); the v
     phase runs column-block-major so field supplies the conv engines early
  4. causal multi-scale dilated conv collapsed to 22 distinct time offsets,
     split across ALL FOUR compute engines with per-engine partial
     accumulators so no engine chain blocks another (see CONV_BANDS):
       - COUPLE_CONV offsets: extra accumulation passes of the head-coupling
         matmul with stationaries G*diag(w_o) (TensorE; adds are free in
         PSUM and need no extra synchronization)
       - acc_dve: init via 4x-mode tensor_scalar, then 2x-mode bf16
         tensor_adds of tmps produced by ScalarE Copy-with-scale ("a") or
         DVE tensor_scalar ("d")
       - acc_pool: init via DVE tensor_scalar, GpSimd/Pool tensor_adds of
         DVE-scaled tmps ("p")
     The coupling matmul takes the G_o passes + G @ acc_dve + G @ acc_pool
     in one PSUM accumulation group.  Conv work is emitted in small bands at
     pacing points through the v phase and the output loop (engines drain
     their queues in order, so emission order is the schedule).
  5. gate = Sigmoid(x @ Wg.T + 2.0) per 512-token segment (fp8 DoubleRow),
     gated = z * gate (gated reuses the x buffer)
  6. out = gated.T @ Wo.T (+ out_b) with gated [c~,n] chunks as the
     stationary operand so the output lands token-major for the DMA out.
The tiny softmaxes (scale gains, coupling rows) are computed on-device; the
scale-gain input arrives pre-transposed/replicated to 128 partitions so the
conv weights w[p, o] need no partition-replication DMAs.  All DMAs use the
HWDGE (sync) queues: SWDGE descriptor generation shares the Q7 core with
Pool tensor ops and the combination crashes the device.
"""

import os
import sys

import numpy as np

# recover wedged NeuronCores from a previously killed process
os.environ.setdefault("NEURON_RT_RESET_CORES", "1")

for _p in ("/opt/trn_rl_repo", "/root/.axon_site/_ro/trn_rl_repo"):
    if _p not in sys.path:
        sys.path.append(_p)

import ml_dtypes  # noqa: E402
import concourse.bass as bass  # noqa: E402
import concourse.tile as tile  # noqa: E402
from concourse import bacc, mybir  # noqa: E402
from concourse import bass_utils  # noqa: E402

BF16 = mybir.dt.bfloat16
F32 = mybir.dt.float32
FP8 = mybir.dt.float8e4
NP_BF16 = ml_dtypes.bfloat16
NP_FP8 = ml_dtypes.float8_e4m3

B, N, D = 8, 2048, 1024
H, HD = 16, 64
S = 11  # scales
NCORES = 8
P = 128  # partitions
CH = D // P  # 8 channel chunks
NT = N // P  # 16 token tiles
NCK = N // 512  # 4 free-dim 512 chunks

D4 = np.array(
    [0.4829629131445341, 0.8365163037378079, 0.2241438680420134, -0.1294095225512604],
    dtype=np.float64,
)

# Distinct causal time offsets (3-t)*2^j < N, and the [n_offsets, S] map s.t.
# w[o, h] = sum_j A_MAP[o, j] * softmax_gains[j, h]
_offs = sorted({(3 - t) * (1 << j) for j in range(S) for t in range(4)} & set(range(N)))
OFFSETS = list(_offs)
NOFF = len(OFFSETS)  # 22
A_MAP = np.zeros((NOFF, S), dtype=np.float64)
for j in range(S):
    for t in range(4):
        o = (3 - t) * (1 << j)
        if o < N:
            A_MAP[OFFSETS.index(o), j] += D4[t]

# d-major channel permutation: c~ -> original feature h*64 + d
PERM = np.array([(c % H) * HD + c // H for c in range(D)], dtype=np.int64)

# conv offset -> engine assignment (cost-model LP balance).  PE-coupling
# passes are the cheapest col-rate AND sync-free, so PE carries offsets
# until it is the clear bottleneck; the assist engines stay under it so the
# per-segment couple barrier never starves.
COUPLE_CONV = (0, 1, 2, 4, 6, 8)  # extra coupling-matmul passes (TensorE)
INIT_OFF = 3              # DVE tensor_scalar init of acc_dve
POOL_INIT = 64            # DVE tensor_scalar init of acc_pool
# bands: small op groups emitted at pacing points throughout the v phase and
# the output loop so every engine has a rolling supply of independent work.
# kinds: i=acc_dve init, a=Act scale + DVE add, p=DVE scale + Pool add,
#        q=acc_pool init, d=DVE scale + DVE add
CONV_BANDS = (
    (("i", 3), ("a", 12), ("a", 16)),
    (("a", 24), ("a", 32)),
    (("a", 48), ("q", 64)),
    (("p", 96), ("p", 128)),
    (("p", 192), ("p", 384)),
    (("d", 256), ("d", 512)),
    (("d", 768), ("d", 1024), ("d", 1536)),
)
SEL_FP8 = True            # fp8 DoubleRow selector matmul for kmag
VCP = 2                   # vc's processed per conv op (wider ops, fewer sems)

_CACHE = {}


def _build_program(iters=1, ob_zero=False):
    nc = bacc.Bacc("TRN2", target_bir_lowering=False, debug=False, num_devices=NCORES)

    # ---- I/O ----
    x_cm = nc.dram_tensor("x_cm", [D, N], BF16, kind="ExternalInput")
    x8f_d = nc.dram_tensor("x8f", [D, N], FP8, kind="ExternalInput")
    wk8_d = nc.dram_tensor("wk8", [D, D], FP8, kind="ExternalInput")  # [c_in, kf]
    wv_d = nc.dram_tensor("wv", [D, D], BF16, kind="ExternalInput")  # [c_in, c~]
    wg8_d = nc.dram_tensor("wg8", [D, D], FP8, kind="ExternalInput")  # [c_in, c~]
    wo_d = nc.dram_tensor("wo", [D, D], BF16, kind="ExternalInput")  # [c~, f]
    bk_d = nc.dram_tensor("bk", [P, CH], F32, kind="ExternalInput")
    bv_d = nc.dram_tensor("bv", [P, CH], F32, kind="ExternalInput")
    bg_d = nc.dram_tensor("bg", [P, CH], F32, kind="ExternalInput")
    ob_d = nc.dram_tensor("ob", [P, D], F32, kind="ExternalInput")  # out_b row-bcast
    sg_d = nc.dram_tensor("sg", [P, S], F32, kind="ExternalInput")
    fc_d = nc.dram_tensor("fc", [H, H], F32, kind="ExternalInput")
    y_d = nc.dram_tensor("y", [N, D], F32, kind="ExternalOutput")

    # ---- constants (embedded in NEFF) ----
    a_rep = np.broadcast_to(A_MAP.T.astype(np.float32), (P, S, NOFF))
    a_rep_d = nc.inline_tensor(np.ascontiguousarray(a_rep), "a_rep")
    if SEL_FP8:
        # sel8[p, j, pr, po] = 1 iff head of k-channel (256*pr + 128*j + p)
        # == po %% 16: the selector matmul then emits kmag^2 already
        # replicated across all 128 partitions (matmul cost is per free
        # column, so the replication is free)
        sel8 = np.zeros((P, 2, CH // 2, P), dtype=NP_FP8)
        for pr in range(CH // 2):
            for j in range(2):
                for p in range(P):
                    h = (256 * pr + 128 * j + p) // HD
                    for r in range(P // H):
                        sel8[p, j, pr, H * r + h] = 1
        sel_d = nc.inline_tensor(np.ascontiguousarray(sel8), "sel8")
    else:
        sel = np.zeros((P, CH, H), dtype=NP_BF16)
        for kc in range(CH):
            for p in range(P):
                sel[p, kc, 2 * kc + p // HD] = 1
        sel_d = nc.inline_tensor(np.ascontiguousarray(sel), "sel")
    i16_d = nc.inline_tensor(np.eye(H, dtype=NP_BF16), "i16")

    init_oi = OFFSETS.index(INIT_OFF)

    import contextlib
    with tile.TileContext(nc) as tc, contextlib.ExitStack() as _st:
      for _it in range(iters):
          with (
              tc.tile_pool(name="consts", bufs=1) as cpool,
              tc.tile_pool(name="xpool", bufs=1) as xpool,
              tc.tile_pool(name="x8pool", bufs=1) as x8pool,
              tc.tile_pool(name="wk8pool", bufs=1) as wk8pool,
              tc.tile_pool(name="wpool", bufs=1) as wpool,
              tc.tile_pool(name="field", bufs=1) as fpool,
              tc.tile_pool(name="accd", bufs=3) as adpool,
              tc.tile_pool(name="accp", bufs=2) as appool,
              tc.tile_pool(name="gatep", bufs=1) as gpool,
              tc.tile_pool(name="k2p", bufs=2) as k2pool,
              tc.tile_pool(name="tmpa", bufs=3) as tmpapool,
              tc.tile_pool(name="tmpd", bufs=3) as tmpdpool,
              tc.tile_pool(name="tmpq", bufs=2) as tmpqpool,
              tc.tile_pool(name="ystg", bufs=2) as ypool,
              tc.tile_pool(name="psum", bufs=4, space="PSUM") as pspool,
              tc.tile_pool(name="psum_km", bufs=2, space="PSUM") as kmpool,
          ):
              # ============ tiny INPUT-ONLY dmas first (no compute deps, so
              # the in-order HWDGE queues never head-of-line block) ========
              sg_sb = cpool.tile([P, S], F32)
              nc.sync.dma_start(out=sg_sb[:, :], in_=sg_d[:, :])
              a_sb = cpool.tile([P, S, NOFF], F32)
              nc.sync.dma_start(out=a_sb[:, :, :], in_=a_rep_d[:, :, :])
              fc_sb = cpool.tile([H, H], F32)
              nc.sync.dma_start(out=fc_sb[:, :], in_=fc_d[:, :])
              i16_sb = cpool.tile([H, H], BF16)
              nc.sync.dma_start(out=i16_sb[:, :], in_=i16_d[:, :])
              if SEL_FP8:
                  sel_sb = cpool.tile([P, 2, CH // 2, P], FP8)
                  nc.sync.dma_start(out=sel_sb[:, :, :, :], in_=sel_d[:, :, :, :])
              else:
                  sel_sb = cpool.tile([P, CH, H], BF16)
                  nc.sync.dma_start(out=sel_sb[:, :, :], in_=sel_d[:, :, :])
              bk_sb = cpool.tile([P, CH], F32)
              nc.sync.dma_start(out=bk_sb[:, :], in_=bk_d[:, :])
              bv_sb = cpool.tile([P, CH], F32)
              nc.sync.dma_start(out=bv_sb[:, :], in_=bv_d[:, :])
              bg_sb = cpool.tile([P, CH], F32)
              nc.sync.dma_start(out=bg_sb[:, :], in_=bg_d[:, :])
              if not ob_zero:
                  ob_sb = cpool.tile([P, D], F32)
                  nc.sync.dma_start(out=ob_sb[:, :], in_=ob_d[:, :])

              # ============ big streaming inputs ============
              x_sb = xpool.tile([P, CH, N], BF16)
              x8_sb = x8pool.tile([P, 4, 2, N], FP8)
              wk8_sb = wk8pool.tile([P, 4, 2, D], FP8)
              for ic in range(4):
                  for j in range(2):
                      r = 256 * ic + 128 * j
                      nc.sync.dma_start(out=wk8_sb[:, ic, j, :],
                                        in_=wk8_d[r:r + P, :])
                      nc.sync.dma_start(out=x8_sb[:, ic, j, :],
                                        in_=x8f_d[r:r + P, :])
              for ic in range(CH):
                  nc.sync.dma_start(out=x_sb[:, ic, :], in_=x_cm[P * ic:P * (ic + 1), :])
              # wv into the shared wv/wo slot: DMA starts immediately and
              # overlaps the whole k phase
              wv_sb = wpool.tile([P, CH, D], BF16, tag="wmat")
              for ic in range(CH):
                  nc.sync.dma_start(out=wv_sb[:, ic, :], in_=wv_d[P * ic:P * (ic + 1), :])
              # gate weights (fp8) live in the const pool so the wpool slot
              # can rotate wv -> wo; needed only from the first output seg
              wg8_sb = cpool.tile([P, 4, 2, D], FP8)
              for ic in range(4):
                  for j in range(2):
                      r = 256 * ic + 128 * j
                      nc.sync.dma_start(out=wg8_sb[:, ic, j, :],
                                        in_=wg8_d[r:r + P, :])

              # ============ derived parameters (compute + SBUF copies that
              # would otherwise block the DMA queues behind their deps) ====
              # softmax of scale_gain over scales, per head -> gains [16, 11]
              sg_mx = cpool.tile([P, 1], F32)
              nc.vector.reduce_max(out=sg_mx[:, :], in_=sg_sb[:, :], axis=mybir.AxisListType.X)
              nc.vector.tensor_scalar_mul(sg_mx[:, :], sg_mx[:, :], -1.0)
              sg_e = cpool.tile([P, S], F32)
              nc.scalar.activation(
                  out=sg_e[:, :], in_=sg_sb[:, :],
                  func=mybir.ActivationFunctionType.Exp, bias=sg_mx[:, 0:1], scale=1.0,
              )
              sg_sum = cpool.tile([P, 1], F32)
              nc.vector.reduce_sum(out=sg_sum[:, :], in_=sg_e[:, :], axis=mybir.AxisListType.X)
              sg_rec = cpool.tile([P, 1], F32)
              nc.vector.reciprocal(out=sg_rec[:, :], in_=sg_sum[:, :])
              gains = cpool.tile([P, S], F32)
              nc.vector.tensor_scalar_mul(gains[:, :], sg_e[:, :], sg_rec[:, 0:1])

              # conv coefficients w[p, o] = sum_j gains[p, j] * A_MAP[o, j],
              # computed on all 128 partitions directly (sg arrives
              # pre-replicated), so no partition-replication DMAs
              w_rep = cpool.tile([P, NOFF], F32)
              nc.vector.tensor_scalar_mul(w_rep[:, :], a_sb[:, 0, :], gains[:, 0:1])
              for j in range(1, S):
                  nc.vector.scalar_tensor_tensor(
                      out=w_rep[:, :], in0=a_sb[:, j, :], scalar=gains[:, j:j + 1],
                      in1=w_rep[:, :], op0=mybir.AluOpType.mult, op1=mybir.AluOpType.add,
                  )

              # coupling softmax (rows) -> C_sm; G = I_8 (x) C_sm^T [128,128]
              fc_mx = cpool.tile([H, 1], F32)
              nc.vector.reduce_max(out=fc_mx[:, :], in_=fc_sb[:, :], axis=mybir.AxisListType.X)
              nc.vector.tensor_scalar_mul(fc_mx[:, :], fc_mx[:, :], -1.0)
              fc_e = cpool.tile([H, H], F32)
              nc.scalar.activation(
                  out=fc_e[:, :], in_=fc_sb[:, :],
                  func=mybir.ActivationFunctionType.Exp, bias=fc_mx[:, 0:1], scale=1.0,
              )
              fc_sum = cpool.tile([H, 1], F32)
              nc.vector.reduce_sum(out=fc_sum[:, :], in_=fc_e[:, :], axis=mybir.AxisListType.X)
              fc_rec = cpool.tile([H, 1], F32)
              nc.vector.reciprocal(out=fc_rec[:, :], in_=fc_sum[:, :])
              csm_bf = cpool.tile([H, H], BF16)
              nc.vector.tensor_scalar_mul(csm_bf[:, :], fc_e[:, :], fc_rec[:, 0:1])
              ct_ps = pspool.tile([H, H], BF16, tag="mm")
              nc.tensor.transpose(out=ct_ps[:, :], in_=csm_bf[:, :], identity=i16_sb[:, :])
              ct_bf = cpool.tile([H, H], BF16)
              nc.vector.tensor_copy(ct_bf[:, :], ct_ps[:, :])
              g_sb = cpool.tile([P, P], BF16)
              nc.vector.memset(g_sb[:, :], 0.0)
              for r in range(CH):
                  nc.sync.dma_start(
                      out=g_sb[H * r:H * (r + 1), H * r:H * (r + 1)], in_=ct_bf[:, :]
                  )
              # G_o = G * diag(w_o) column-scaled stationaries for the conv
              # offsets folded into the coupling matmul: G_o[c', c] =
              # G[c', c] * w_o[c']  (per-partition scale)
              gcoup = cpool.tile([P, len(COUPLE_CONV), P], BF16)
              for gi, o in enumerate(COUPLE_CONV):
                  oi = OFFSETS.index(o)
                  nc.vector.tensor_scalar_mul(
                      gcoup[:, gi, :], g_sb[:, :], w_rep[:, oi:oi + 1]
                  )

              # ============ k phase: kmag_rep[p, n] = kmag[p%16, n] ========
              kmag_rep = cpool.tile([P, N], BF16)

              def k_proj_chunk(kc, ns, ps):
                  for ic in range(4):
                      nc.tensor.matmul(
                          ps[:, :],
                          lhsT=wk8_sb[:, ic, :, P * kc:P * (kc + 1)],
                          rhs=x8_sb[:, ic, :, ns:ns + 512],
                          perf_mode=mybir.MatmulPerfMode.DoubleRow,
                          start=(ic == 0), stop=(ic == 3),
                      )

              def k_phase_chunk(nch):
                  ns = 512 * nch
                  km_ps = kmpool.tile([P, 512], F32, tag="km")
                  if SEL_FP8:
                      for pr in range(CH // 2):
                          k2 = k2pool.tile([P, 2, 512], FP8, tag="k2")
                          for j in range(2):
                              kc = 2 * pr + j
                              ps = pspool.tile([P, 512], F32, tag="mm")
                              k_proj_chunk(kc, ns, ps)
                              nc.scalar.activation(
                                  out=k2[:, j, :], in_=ps[:, :],
                                  func=mybir.ActivationFunctionType.Square,
                                  bias=bk_sb[:, kc:kc + 1], scale=1.0,
                              )
                          nc.tensor.matmul(
                              km_ps[:, :],
                              lhsT=sel_sb[:, :, pr, :], rhs=k2[:, :, :],
                              perf_mode=mybir.MatmulPerfMode.DoubleRow,
                              start=(pr == 0), stop=(pr == CH // 2 - 1),
                          )
                  else:
                      for kc in range(CH):
                          ps = pspool.tile([P, 512], F32, tag="mm")
                          k_proj_chunk(kc, ns, ps)
                          k2 = k2pool.tile([P, 512], BF16, tag="k2")
                          nc.scalar.activation(
                              out=k2[:, :], in_=ps[:, :],
                              func=mybir.ActivationFunctionType.Square,
                              bias=bk_sb[:, kc:kc + 1], scale=1.0,
                          )
                          nc.tensor.matmul(
                              km_ps[:, :],
                              lhsT=sel_sb[:, kc, :], rhs=k2[:, :],
                              start=(kc == 0), stop=(kc == CH - 1),
                          )
                  nc.scalar.activation(
                      out=kmag_rep[:, ns:ns + 512], in_=km_ps[:, :],
                      func=mybir.ActivationFunctionType.Sqrt,
                  )

              def kmag_replicate(nch):
                  ns = 512 * nch
                  for r in range(1, P // H):
                      nc.sync.dma_start(
                          out=kmag_rep[H * r:H * (r + 1), ns:ns + 512],
                          in_=kmag_rep[0:H, ns:ns + 512])

              # ============ v phase (nch-major: field completes by column
              # blocks so the conv engines get supply early) ============
              field = fpool.tile([P, CH, N], BF16)

              def v_chunk(nch):
                  ns = 512 * nch
                  for vc in range(CH):
                      ps = pspool.tile([P, 512], F32, tag="mm")
                      for ic in range(CH):
                          nc.tensor.matmul(
                              ps[:, :],
                              lhsT=wv_sb[:, ic, P * vc:P * (vc + 1)],
                              rhs=x_sb[:, ic, ns:ns + 512],
                              start=(ic == 0), stop=(ic == CH - 1),
                          )
                      nc.vector.scalar_tensor_tensor(
                          out=field[:, vc, ns:ns + 512],
                          in0=ps[:, :], scalar=bv_sb[:, vc:vc + 1],
                          in1=kmag_rep[:, ns:ns + 512],
                          op0=mybir.AluOpType.add, op1=mybir.AluOpType.mult,
                      )

              # wo into the slot freed by wv (DMA overlaps conv/gate)
              wo_sb = wpool.tile([P, CH, D], BF16, tag="wmat")
              for ic in range(CH):
                  nc.sync.dma_start(out=wo_sb[:, ic, :], in_=wo_d[P * ic:P * (ic + 1), :])

              acc_dve = {}
              acc_pool = {}
              NQ = CH // VCP  # vc-pair groups per op

              def conv_band(seg, band):
                  if seg >= NCK:
                      return
                  ns = 512 * seg
                  if band == 0:
                      acc_dve[seg] = adpool.tile([P, CH, 512], BF16, tag="ad",
                                                 name=f"ad{seg}")
                      acc_pool[seg] = appool.tile([P, CH, 512], BF16, tag="ap",
                                                  name=f"ap{seg}")
                  ad, ap_ = acc_dve[seg], acc_pool[seg]
                  for kind, o in CONV_BANDS[band]:
                      lo = max(ns, o)
                      if lo >= ns + 512:
                          continue
                      cs = lo - ns  # start col within the segment
                      w = 512 - cs
                      oi = OFFSETS.index(o)
                      vw = VCP if kind in ("p", "q") else 2 * VCP
                      for q in range(CH // vw):
                          v0, v1 = vw * q, vw * (q + 1)
                          src = field[:, v0:v1, lo - o:ns + 512 - o]
                          if kind == "i":
                              nc.vector.tensor_scalar_mul(
                                  ad[:, v0:v1, cs:512], src, w_rep[:, oi:oi + 1])
                          elif kind == "q":
                              nc.vector.tensor_scalar_mul(
                                  ap_[:, v0:v1, cs:512], src, w_rep[:, oi:oi + 1])
                          elif kind == "a":
                              tmp = tmpapool.tile([P, 2 * VCP, 512], BF16, tag="tmpa")
                              nc.scalar.activation(
                                  out=tmp[:, :, 0:w], in_=src,
                                  func=mybir.ActivationFunctionType.Copy,
                                  scale=w_rep[:, oi:oi + 1],
                              )
                              nc.vector.tensor_add(
                                  ad[:, v0:v1, cs:512], ad[:, v0:v1, cs:512],
                                  tmp[:, :, 0:w])
                          elif kind == "p":
                              tmp = tmpqpool.tile([P, VCP, 512], BF16, tag="tmpq")
                              nc.vector.tensor_scalar_mul(
                                  tmp[:, :, 0:w], src, w_rep[:, oi:oi + 1])
                              if os.environ.get("DBG_NO_POOL"):
                                  nc.vector.tensor_add(
                                      ap_[:, v0:v1, cs:512],
                                      ap_[:, v0:v1, cs:512], tmp[:, :, 0:w])
                              else:
                                  nc.gpsimd.tensor_add(
                                      ap_[:, v0:v1, cs:512],
                                      ap_[:, v0:v1, cs:512], tmp[:, :, 0:w])
                          else:  # "d": DVE scale + DVE add
                              tmp = tmpdpool.tile([P, 2 * VCP, 512], BF16,
                                                  tag="tmpd", name="tmpw")
                              nc.vector.tensor_scalar_mul(
                                  tmp[:, :, 0:w], src, w_rep[:, oi:oi + 1])
                              nc.vector.tensor_add(
                                  ad[:, v0:v1, cs:512], ad[:, v0:v1, cs:512],
                                  tmp[:, :, 0:w])

              def gate_seg(seg, gate):
                  ns = 512 * seg
                  for gc in range(CH):
                      ps = pspool.tile([P, 512], F32, tag="mm")
                      for ic in range(4):
                          nc.tensor.matmul(
                              ps[:, :],
                              lhsT=wg8_sb[:, ic, :, P * gc:P * (gc + 1)],
                              rhs=x8_sb[:, ic, :, ns:ns + 512],
                              perf_mode=mybir.MatmulPerfMode.DoubleRow,
                              start=(ic == 0), stop=(ic == 3),
                          )
                      nc.scalar.activation(
                          out=gate[:, gc, :], in_=ps[:, :],
                          func=mybir.ActivationFunctionType.Sigmoid,
                          bias=bg_sb[:, gc:gc + 1], scale=1.0,
                      )

              # band schedule: conv(s) only needs field column blocks <= s,
              # so bands stream out at pacing points starting inside the v
              # phase.  acc pool rotation throttles how far ahead this runs.
              NB = len(CONV_BANDS)
              sched = [(s, b) for s in range(NCK) for b in range(NB)]
              spos = [0]

              def emit_bands(k):
                  while k > 0 and spos[0] < len(sched):
                      s, b = sched[spos[0]]
                      spos[0] += 1
                      conv_band(s, b)
                      k -= 1


              for _nch in range(NCK):
                  k_phase_chunk(_nch)
              v_chunk(0)
              emit_bands(2)
              v_chunk(1)
              emit_bands(4)
              v_chunk(2)
              emit_bands(4)
              # gate(0) hoisted into the v phase: its sigmoids land early in
              # the Act queue so the first output segment's psum slots free
              # promptly
              gate0 = gpool.tile([P, CH, 512], BF16, tag="gate")
              gate_seg(0, gate0)
              v_chunk(3)
              emit_bands(4)

              # ========= per-segment: gate, coupling + gate-mul, out proj ==
              gated = x_sb  # dead after the v phase; reused in d-major layout

              def couple_seg(seg, gate):
                  ns = 512 * seg
                  ad, ap_ = acc_dve.pop(seg), acc_pool.pop(seg)
                  lo0 = max(0, INIT_OFF - ns)
                  lop = max(0, POOL_INIT - ns)
                  # merge the Pool partial into acc_dve (2x-mode adds) so the
                  # coupling needs one acc pass instead of two
                  for q in range(2):
                      v0, v1 = 4 * q, 4 * (q + 1)
                      nc.vector.tensor_add(
                          ad[:, v0:v1, lop:512], ad[:, v0:v1, lop:512],
                          ap_[:, v0:v1, lop:512])
                  for vc in range(CH):
                      ps = pspool.tile([P, 512], F32, tag="mm")
                      for gi, o in enumerate(COUPLE_CONV):
                          lo = max(0, o - ns)
                          # gi == 0 is offset 0 (full width) -> start pass
                          nc.tensor.matmul(
                              ps[:, lo:512],
                              lhsT=gcoup[:, gi, :],
                              rhs=field[:, vc, ns + lo - o:ns + 512 - o],
                              start=(gi == 0), stop=False,
                          )
                      nc.tensor.matmul(
                          ps[:, lo0:512], lhsT=g_sb[:, :], rhs=ad[:, vc, lo0:512],
                          start=False, stop=True,
                      )
                      nc.vector.tensor_mul(
                          gated[:, vc, ns:ns + 512], ps[:, :], gate[:, vc, :],
                      )

              def out_tile(nt):
                  for fch in range(2):
                      fs = 512 * fch
                      ps = pspool.tile([P, 512], F32, tag="mm")
                      for vc in range(CH):
                          nc.tensor.matmul(
                              ps[:, :],
                              lhsT=gated[:, vc, P * nt:P * (nt + 1)],
                              rhs=wo_sb[:, vc, fs:fs + 512],
                              start=(vc == 0), stop=(vc == CH - 1),
                          )
                      ystg = ypool.tile([P, 512], F32, tag="y")
                      if ob_zero:
                          nc.scalar.activation(
                              out=ystg[:, :], in_=ps[:, :],
                              func=mybir.ActivationFunctionType.Copy,
                          )
                      else:
                          nc.vector.tensor_add(
                              ystg[:, :], ps[:, :], ob_sb[:, fs:fs + 512],
                          )
                      nc.sync.dma_start(out=y_d[P * nt:P * (nt + 1), fs:fs + 512],
                                        in_=ystg[:, :])

              for seg in range(NCK):
                  if seg == 0:
                      gate = gate0
                  else:
                      gate = gpool.tile([P, CH, 512], BF16, tag="gate")
                      gate_seg(seg, gate)
                  couple_seg(seg, gate)
                  emit_bands(2)
                  for nt in range(4 * seg, 4 * seg + 4):
                      out_tile(nt)
                      emit_bands(1)

    nc.compile()
    return nc


def _prep_shared(qkv_w, qkv_b, out_w, out_b, gate_w, gate_b, scale_gain, field_coupling):
    perm = PERM
    wk8 = np.ascontiguousarray(qkv_w[D:2 * D, :].T.astype(NP_FP8))
    wv = np.ascontiguousarray(qkv_w[2 * D:3 * D, :][perm, :].T.astype(NP_BF16))
    wg8 = np.ascontiguousarray(gate_w[perm, :].T.astype(NP_FP8))
    wo = np.ascontiguousarray(out_w[:, perm].T.astype(NP_BF16))
    bk = np.ascontiguousarray(qkv_b[D:2 * D].reshape(CH, P).T.astype(np.float32))
    bv = np.ascontiguousarray(qkv_b[2 * D:3 * D][perm].reshape(CH, P).T.astype(np.float32))
    bg = np.ascontiguousarray(gate_b[perm].reshape(CH, P).T.astype(np.float32))
    ob = np.ascontiguousarray(np.broadcast_to(out_b.astype(np.float32), (P, D)))
    sg = np.ascontiguousarray(np.tile(scale_gain.T.astype(np.float32),
                                      (P // H, 1)))
    fc = np.ascontiguousarray(field_coupling.astype(np.float32))
    return {"wk8": wk8, "wv": wv, "wg8": wg8, "wo": wo, "bk": bk,
            "bv": bv, "bg": bg, "ob": ob, "sg": sg, "fc": fc}


def _make_in_maps(x, shared):
    in_maps = []
    for b in range(B):
        m = dict(shared)
        xt = np.ascontiguousarray(x[b].T)
        m["x_cm"] = xt.astype(NP_BF16)
        m["x8f"] = xt.astype(NP_FP8)
        in_maps.append(m)
    return in_maps


def kernel(x, qkv_w, qkv_b, out_w, out_b, gate_w, gate_b, scale_gain,
           field_coupling):
    x = np.asarray(x, dtype=np.float32)
    qkv_w = np.asarray(qkv_w, dtype=np.float32)
    qkv_b = np.asarray(qkv_b, dtype=np.float32)
    out_w = np.asarray(out_w, dtype=np.float32)
    out_b = np.asarray(out_b, dtype=np.float32)
    gate_w = np.asarray(gate_w, dtype=np.float32)
    gate_b = np.asarray(gate_b, dtype=np.float32)
    scale_gain = np.asarray(scale_gain, dtype=np.float32)
    field_coupling = np.asarray(field_coupling, dtype=np.float32)

    ob_zero = not np.any(out_b)
    key = ("nc", ob_zero)
    if key not in _CACHE:
        _CACHE[key] = _build_program(ob_zero=ob_zero)
    nc = _CACHE[key]

    shared = _prep_shared(qkv_w, qkv_b, out_w, out_b, gate_w, gate_b,
                          scale_gain, field_coupling)
    in_maps = _make_in_maps(x, shared)

    res = bass_utils.run_bass_kernel_spmd(nc, in_maps, list(range(NCORES)))
    out = np.stack([np.asarray(res.results[b]["y"], dtype=np.float32)
                    for b in range(B)], axis=0)
    return out


# revision 84
# speedup vs baseline: 1.3371x; 1.3371x over previous
"""Trainium2 Bass kernel for CausalWaveletFieldAttention.

Full-input contract: kernel(**inputs) takes the complete (unsharded) numpy
inputs and returns the full [8, 2048, 1024] float32 output.

Sharding: pure data-parallel over batch B=8 -> one batch element per
NeuronCore (8 cores), zero collectives (the head-coupling einsum mixes heads
within a batch element only).

Per-core pipeline (x pre-transposed to feature-major on host, bf16 compute,
fp32 PSUM accumulation, fp8 DoubleRow for the k and gate projections):
  1. k = x @ Wk.T       (TensorE, fp8 DoubleRow; contraction index
     c = 256*ic + 128*j + p so the fp8 operands are plain row-slices of the
     feature-major [D, N] fp8 copy of x -- no host interleave needed)
  2. k2 = Square(k + bk) (ScalarE, fp8 output), per-head sums of 64
     partitions via an fp8 DoubleRow selector matmul (TensorE) ->
     kmag = sqrt(.) (ScalarE)
  3. v = x @ Wv.T with v output channels permuted to d-major order
     (channel c~ = d*16 + h); field = (v + bv) * kmag (fused DVE op); the v
     phase runs column-block-major so field supplies the conv engines early
  4. causal multi-scale dilated conv collapsed to 22 distinct time offsets,
     split across ALL FOUR compute engines with per-engine partial
     accumulators so no engine chain blocks another (see CONV_BANDS):
       - COUPLE_CONV offsets: extra accumulation passes of the head-coupling
         matmul with stationaries G*diag(w_o) (TensorE; adds are free in
         PSUM and need no extra synchronization)
       - acc_dve: init via 4x-mode tensor_scalar, then 2x-mode bf16
         tensor_adds of tmps produced by ScalarE Copy-with-scale ("a") or
         DVE tensor_scalar ("d")
       - acc_pool: init via DVE tensor_scalar, GpSimd/Pool tensor_adds of
         DVE-scaled tmps ("p")
     The coupling matmul takes the G_o passes + G @ acc_dve + G @ acc_pool
     in one PSUM accumulation group.  Conv work is emitted in small bands at
     pacing points through the v phase and the output loop (engines drain
     their queues in order, so emission order is the schedule).
  5. gate = Sigmoid(x @ Wg.T + 2.0) per 512-token segment (fp8 DoubleRow),
     gated = z * gate (gated reuses the x buffer)
  6. out = gated.T @ Wo.T (+ out_b) with gated [c~,n] chunks as the
     stationary operand so the output lands token-major for the DMA out.
The tiny softmaxes (scale gains, coupling rows) are computed on-device; the
scale-gain input arrives pre-transposed/replicated to 128 partitions so the
conv weights w[p, o] need no partition-replication DMAs.  All DMAs use the
HWDGE (sync) queues: SWDGE descriptor generation shares the Q7 core with
Pool tensor ops and the combination crashes the device.
"""

import os
import sys

import numpy as np

# recover wedged NeuronCores from a previously killed process
os.environ.setdefault("NEURON_RT_RESET_CORES", "1")

for _p in ("/opt/trn_rl_repo", "/root/.axon_site/_ro/trn_rl_repo"):
    if _p not in sys.path:
        sys.path.append(_p)

import ml_dtypes  # noqa: E402
import concourse.bass as bass  # noqa: E402
import concourse.tile as tile  # noqa: E402
from concourse import bacc, mybir  # noqa: E402
from concourse import bass_utils  # noqa: E402

BF16 = mybir.dt.bfloat16
F32 = mybir.dt.float32
FP8 = mybir.dt.float8e4
NP_BF16 = ml_dtypes.bfloat16
NP_FP8 = ml_dtypes.float8_e4m3

B, N, D = 8, 2048, 1024
H, HD = 16, 64
S = 11  # scales
NCORES = 8
P = 128  # partitions
CH = D // P  # 8 channel chunks
NT = N // P  # 16 token tiles
NCK = N // 512  # 4 free-dim 512 chunks

D4 = np.array(
    [0.4829629131445341, 0.8365163037378079, 0.2241438680420134, -0.1294095225512604],
    dtype=np.float64,
)

# Distinct causal time offsets (3-t)*2^j < N, and the [n_offsets, S] map s.t.
# w[o, h] = sum_j A_MAP[o, j] * softmax_gains[j, h]
_offs = sorted({(3 - t) * (1 << j) for j in range(S) for t in range(4)} & set(range(N)))
OFFSETS = list(_offs)
NOFF = len(OFFSETS)  # 22
A_MAP = np.zeros((NOFF, S), dtype=np.float64)
for j in range(S):
    for t in range(4):
        o = (3 - t) * (1 << j)
        if o < N:
            A_MAP[OFFSETS.index(o), j] += D4[t]

# d-major channel permutation: c~ -> original feature h*64 + d
PERM = np.array([(c % H) * HD + c // H for c in range(D)], dtype=np.int64)

# conv offset -> engine assignment (cost-model LP balance).  PE-coupling
# passes are the cheapest col-rate AND sync-free, so PE carries offsets
# until it is the clear bottleneck; the assist engines stay under it so the
# per-segment couple barrier never starves.
COUPLE_CONV = (0, 1, 2, 4, 6, 8)  # extra coupling-matmul passes (TensorE)
INIT_OFF = 3              # DVE tensor_scalar init of acc_dve
POOL_INIT = 64            # DVE tensor_scalar init of acc_pool
# bands: small op groups emitted at pacing points throughout the v phase and
# the output loop so every engine has a rolling supply of independent work.
# kinds: i=acc_dve init, a=Act scale + DVE add, p=DVE scale + Pool add,
#        q=acc_pool init, d=DVE scale + DVE add
CONV_BANDS = (
    (("i", 3), ("a", 12), ("a", 16)),
    (("a", 24), ("a", 32)),
    (("a", 48), ("q", 64)),
    (("p", 96), ("p", 128)),
    (("p", 192), ("p", 384)),
    (("d", 256), ("d", 512)),
    (("d", 768), ("d", 1024), ("d", 1536)),
)
SEL_FP8 = True            # fp8 DoubleRow selector matmul for kmag
VCP = 2                   # vc's processed per conv op (wider ops, fewer sems)

_CACHE = {}


def _build_program(iters=1, ob_zero=False):
    nc = bacc.Bacc("TRN2", target_bir_lowering=False, debug=False, num_devices=NCORES)

    # ---- I/O ----
    x_cm = nc.dram_tensor("x_cm", [D, N], BF16, kind="ExternalInput")
    x8f_d = nc.dram_tensor("x8f", [D, N], FP8, kind="ExternalInput")
    wk8_d = nc.dram_tensor("wk8", [D, D], FP8, kind="ExternalInput")  # [c_in, kf]
    wv_d = nc.dram_tensor("wv", [D, D], BF16, kind="ExternalInput")  # [c_in, c~]
    wg8_d = nc.dram_tensor("wg8", [D, D], FP8, kind="ExternalInput")  # [c_in, c~]
    wo_d = nc.dram_tensor("wo", [D, D], BF16, kind="ExternalInput")  # [c~, f]
    bk_d = nc.dram_tensor("bk", [P, CH], F32, kind="ExternalInput")
    bv_d = nc.dram_tensor("bv", [P, CH], F32, kind="ExternalInput")
    bg_d = nc.dram_tensor("bg", [P, CH], F32, kind="ExternalInput")
    ob_d = nc.dram_tensor("ob", [P, D], F32, kind="ExternalInput")  # out_b row-bcast
    sg_d = nc.dram_tensor("sg", [P, S], F32, kind="ExternalInput")
    fc_d = nc.dram_tensor("fc", [H, H], F32, kind="ExternalInput")
    y_d = nc.dram_tensor("y", [N, D], F32, kind="ExternalOutput")

    # ---- constants (embedded in NEFF) ----
    a_rep = np.broadcast_to(A_MAP.T.astype(np.float32), (P, S, NOFF))
    a_rep_d = nc.inline_tensor(np.ascontiguousarray(a_rep), "a_rep")
    if SEL_FP8:
        # sel8[p, j, pr, po] = 1 iff head of k-channel (256*pr + 128*j + p)
        # == po %% 16: the selector matmul then emits kmag^2 already
        # replicated across all 128 partitions (matmul cost is per free
        # column, so the replication is free)
        sel8 = np.zeros((P, 2, CH // 2, P), dtype=NP_FP8)
        for pr in range(CH // 2):
            for j in range(2):
                for p in range(P):
                    h = (256 * pr + 128 * j + p) // HD
                    for r in range(P // H):
                        sel8[p, j, pr, H * r + h] = 1
        sel_d = nc.inline_tensor(np.ascontiguousarray(sel8), "sel8")
    else:
        sel = np.zeros((P, CH, H), dtype=NP_BF16)
        for kc in range(CH):
            for p in range(P):
                sel[p, kc, 2 * kc + p // HD] = 1
        sel_d = nc.inline_tensor(np.ascontiguousarray(sel), "sel")
    i16_d = nc.inline_tensor(np.eye(H, dtype=NP_BF16), "i16")

    init_oi = OFFSETS.index(INIT_OFF)

    import contextlib
    with tile.TileContext(nc) as tc, contextlib.ExitStack() as _st:
      for _it in range(iters):
          with (
              tc.tile_pool(name="consts", bufs=1) as cpool,
              tc.tile_pool(name="xpool", bufs=1) as xpool,
              tc.tile_pool(name="x8pool", bufs=1) as x8pool,
              tc.tile_pool(name="wk8pool", bufs=1) as wk8pool,
              tc.tile_pool(name="wpool", bufs=1) as wpool,
              tc.tile_pool(name="field", bufs=1) as fpool,
              tc.tile_pool(name="accd", bufs=3) as adpool,
              tc.tile_pool(name="accp", bufs=2) as appool,
              tc.tile_pool(name="gatep", bufs=1) as gpool,
              tc.tile_pool(name="k2p", bufs=2) as k2pool,
              tc.tile_pool(name="tmpa", bufs=3) as tmpapool,
              tc.tile_pool(name="tmpd", bufs=3) as tmpdpool,
              tc.tile_pool(name="tmpq", bufs=2) as tmpqpool,
              tc.tile_pool(name="ystg", bufs=3) as ypool,
              tc.tile_pool(name="psum", bufs=4, space="PSUM") as pspool,
              tc.tile_pool(name="psum_km", bufs=2, space="PSUM") as kmpool,
          ):
              # ============ tiny INPUT-ONLY dmas first (no compute deps, so
              # the in-order HWDGE queues never head-of-line block) ========
              sg_sb = cpool.tile([P, S], F32)
              nc.sync.dma_start(out=sg_sb[:, :], in_=sg_d[:, :])
              a_sb = cpool.tile([P, S, NOFF], F32)
              nc.sync.dma_start(out=a_sb[:, :, :], in_=a_rep_d[:, :, :])
              fc_sb = cpool.tile([H, H], F32)
              nc.sync.dma_start(out=fc_sb[:, :], in_=fc_d[:, :])
              i16_sb = cpool.tile([H, H], BF16)
              nc.sync.dma_start(out=i16_sb[:, :], in_=i16_d[:, :])
              if SEL_FP8:
                  sel_sb = cpool.tile([P, 2, CH // 2, P], FP8)
                  nc.sync.dma_start(out=sel_sb[:, :, :, :], in_=sel_d[:, :, :, :])
              else:
                  sel_sb = cpool.tile([P, CH, H], BF16)
                  nc.sync.dma_start(out=sel_sb[:, :, :], in_=sel_d[:, :, :])
              bk_sb = cpool.tile([P, CH], F32)
              nc.sync.dma_start(out=bk_sb[:, :], in_=bk_d[:, :])
              bv_sb = cpool.tile([P, CH], F32)
              nc.sync.dma_start(out=bv_sb[:, :], in_=bv_d[:, :])
              bg_sb = cpool.tile([P, CH], F32)
              nc.sync.dma_start(out=bg_sb[:, :], in_=bg_d[:, :])
              if not ob_zero:
                  ob_sb = cpool.tile([P, D], F32)
                  nc.sync.dma_start(out=ob_sb[:, :], in_=ob_d[:, :])

              # ============ big streaming inputs ============
              x_sb = xpool.tile([P, CH, N], BF16)
              x8_sb = x8pool.tile([P, 4, 2, N], FP8)
              wk8_sb = wk8pool.tile([P, 4, 2, D], FP8)
              for ic in range(4):
                  for j in range(2):
                      r = 256 * ic + 128 * j
                      nc.sync.dma_start(out=wk8_sb[:, ic, j, :],
                                        in_=wk8_d[r:r + P, :])
                      nc.sync.dma_start(out=x8_sb[:, ic, j, :],
                                        in_=x8f_d[r:r + P, :])
              for ic in range(CH):
                  nc.sync.dma_start(out=x_sb[:, ic, :], in_=x_cm[P * ic:P * (ic + 1), :])
              # wv into the shared wv/wo slot: DMA starts immediately and
              # overlaps the whole k phase
              wv_sb = wpool.tile([P, CH, D], BF16, tag="wmat")
              for ic in range(CH):
                  nc.sync.dma_start(out=wv_sb[:, ic, :], in_=wv_d[P * ic:P * (ic + 1), :])
              # gate weights (fp8) live in the const pool so the wpool slot
              # can rotate wv -> wo; needed only from the first output seg
              wg8_sb = cpool.tile([P, 4, 2, D], FP8)
              for ic in range(4):
                  for j in range(2):
                      r = 256 * ic + 128 * j
                      nc.sync.dma_start(out=wg8_sb[:, ic, j, :],
                                        in_=wg8_d[r:r + P, :])

              # ============ derived parameters (compute + SBUF copies that
              # would otherwise block the DMA queues behind their deps) ====
              # softmax of scale_gain over scales, per head -> gains [16, 11]
              sg_mx = cpool.tile([P, 1], F32)
              nc.vector.reduce_max(out=sg_mx[:, :], in_=sg_sb[:, :], axis=mybir.AxisListType.X)
              nc.vector.tensor_scalar_mul(sg_mx[:, :], sg_mx[:, :], -1.0)
              sg_e = cpool.tile([P, S], F32)
              nc.scalar.activation(
                  out=sg_e[:, :], in_=sg_sb[:, :],
                  func=mybir.ActivationFunctionType.Exp, bias=sg_mx[:, 0:1], scale=1.0,
              )
              sg_sum = cpool.tile([P, 1], F32)
              nc.vector.reduce_sum(out=sg_sum[:, :], in_=sg_e[:, :], axis=mybir.AxisListType.X)
              sg_rec = cpool.tile([P, 1], F32)
              nc.vector.reciprocal(out=sg_rec[:, :], in_=sg_sum[:, :])
              gains = cpool.tile([P, S], F32)
              nc.vector.tensor_scalar_mul(gains[:, :], sg_e[:, :], sg_rec[:, 0:1])

              # conv coefficients w[p, o] = sum_j gains[p, j] * A_MAP[o, j],
              # computed on all 128 partitions directly (sg arrives
              # pre-replicated), so no partition-replication DMAs
              w_rep = cpool.tile([P, NOFF], F32)
              nc.vector.tensor_scalar_mul(w_rep[:, :], a_sb[:, 0, :], gains[:, 0:1])
              for j in range(1, S):
                  nc.vector.scalar_tensor_tensor(
                      out=w_rep[:, :], in0=a_sb[:, j, :], scalar=gains[:, j:j + 1],
                      in1=w_rep[:, :], op0=mybir.AluOpType.mult, op1=mybir.AluOpType.add,
                  )

              # coupling softmax (rows) -> C_sm; G = I_8 (x) C_sm^T [128,128]
              fc_mx = cpool.tile([H, 1], F32)
              nc.vector.reduce_max(out=fc_mx[:, :], in_=fc_sb[:, :], axis=mybir.AxisListType.X)
              nc.vector.tensor_scalar_mul(fc_mx[:, :], fc_mx[:, :], -1.0)
              fc_e = cpool.tile([H, H], F32)
              nc.scalar.activation(
                  out=fc_e[:, :], in_=fc_sb[:, :],
                  func=mybir.ActivationFunctionType.Exp, bias=fc_mx[:, 0:1], scale=1.0,
              )
              fc_sum = cpool.tile([H, 1], F32)
              nc.vector.reduce_sum(out=fc_sum[:, :], in_=fc_e[:, :], axis=mybir.AxisListType.X)
              fc_rec = cpool.tile([H, 1], F32)
              nc.vector.reciprocal(out=fc_rec[:, :], in_=fc_sum[:, :])
              csm_bf = cpool.tile([H, H], BF16)
              nc.vector.tensor_scalar_mul(csm_bf[:, :], fc_e[:, :], fc_rec[:, 0:1])
              ct_ps = pspool.tile([H, H], BF16, tag="mm")
              nc.tensor.transpose(out=ct_ps[:, :], in_=csm_bf[:, :], identity=i16_sb[:, :])
              ct_bf = cpool.tile([H, H], BF16)
              nc.vector.tensor_copy(ct_bf[:, :], ct_ps[:, :])
              g_sb = cpool.tile([P, P], BF16)
              nc.vector.memset(g_sb[:, :], 0.0)
              for r in range(CH):
                  nc.sync.dma_start(
                      out=g_sb[H * r:H * (r + 1), H * r:H * (r + 1)], in_=ct_bf[:, :]
                  )
              # G_o = G * diag(w_o) column-scaled stationaries for the conv
              # offsets folded into the coupling matmul: G_o[c', c] =
              # G[c', c] * w_o[c']  (per-partition scale)
              gcoup = cpool.tile([P, len(COUPLE_CONV), P], BF16)
              for gi, o in enumerate(COUPLE_CONV):
                  oi = OFFSETS.index(o)
                  nc.vector.tensor_scalar_mul(
                      gcoup[:, gi, :], g_sb[:, :], w_rep[:, oi:oi + 1]
                  )

              # ============ k phase: kmag_rep[p, n] = kmag[p%16, n] ========
              kmag_rep = cpool.tile([P, N], BF16)

              def k_proj_chunk(kc, ns, ps):
                  for ic in range(4):
                      nc.tensor.matmul(
                          ps[:, :],
                          lhsT=wk8_sb[:, ic, :, P * kc:P * (kc + 1)],
                          rhs=x8_sb[:, ic, :, ns:ns + 512],
                          perf_mode=mybir.MatmulPerfMode.DoubleRow,
                          start=(ic == 0), stop=(ic == 3),
                      )

              def k_phase_chunk(nch):
                  ns = 512 * nch
                  km_ps = kmpool.tile([P, 512], F32, tag="km")
                  if SEL_FP8:
                      for pr in range(CH // 2):
                          k2 = k2pool.tile([P, 2, 512], FP8, tag="k2")
                          for j in range(2):
                              kc = 2 * pr + j
                              ps = pspool.tile([P, 512], F32, tag="mm")
                              k_proj_chunk(kc, ns, ps)
                              nc.scalar.activation(
                                  out=k2[:, j, :], in_=ps[:, :],
                                  func=mybir.ActivationFunctionType.Square,
                                  bias=bk_sb[:, kc:kc + 1], scale=1.0,
                              )
                          nc.tensor.matmul(
                              km_ps[:, :],
                              lhsT=sel_sb[:, :, pr, :], rhs=k2[:, :, :],
                              perf_mode=mybir.MatmulPerfMode.DoubleRow,
                              start=(pr == 0), stop=(pr == CH // 2 - 1),
                          )
                  else:
                      for kc in range(CH):
                          ps = pspool.tile([P, 512], F32, tag="mm")
                          k_proj_chunk(kc, ns, ps)
                          k2 = k2pool.tile([P, 512], BF16, tag="k2")
                          nc.scalar.activation(
                              out=k2[:, :], in_=ps[:, :],
                              func=mybir.ActivationFunctionType.Square,
                              bias=bk_sb[:, kc:kc + 1], scale=1.0,
                          )
                          nc.tensor.matmul(
                              km_ps[:, :],
                              lhsT=sel_sb[:, kc, :], rhs=k2[:, :],
                              start=(kc == 0), stop=(kc == CH - 1),
                          )
                  nc.scalar.activation(
                      out=kmag_rep[:, ns:ns + 512], in_=km_ps[:, :],
                      func=mybir.ActivationFunctionType.Sqrt,
                  )

              def kmag_replicate(nch):
                  ns = 512 * nch
                  for r in range(1, P // H):
                      nc.sync.dma_start(
                          out=kmag_rep[H * r:H * (r + 1), ns:ns + 512],
                          in_=kmag_rep[0:H, ns:ns + 512])

              # ============ v phase (nch-major: field completes by column
              # blocks so the conv engines get supply early) ============
              field = fpool.tile([P, CH, N], BF16)

              def v_chunk(nch):
                  ns = 512 * nch
                  for vc in range(CH):
                      ps = pspool.tile([P, 512], F32, tag="mm")
                      for ic in range(CH):
                          nc.tensor.matmul(
                              ps[:, :],
                              lhsT=wv_sb[:, ic, P * vc:P * (vc + 1)],
                              rhs=x_sb[:, ic, ns:ns + 512],
                              start=(ic == 0), stop=(ic == CH - 1),
                          )
                      nc.vector.scalar_tensor_tensor(
                          out=field[:, vc, ns:ns + 512],
                          in0=ps[:, :], scalar=bv_sb[:, vc:vc + 1],
                          in1=kmag_rep[:, ns:ns + 512],
                          op0=mybir.AluOpType.add, op1=mybir.AluOpType.mult,
                      )

              # wo into the slot freed by wv (DMA overlaps conv/gate)
              wo_sb = wpool.tile([P, CH, D], BF16, tag="wmat")
              for ic in range(CH):
                  nc.sync.dma_start(out=wo_sb[:, ic, :], in_=wo_d[P * ic:P * (ic + 1), :])

              acc_dve = {}
              acc_pool = {}
              NQ = CH // VCP  # vc-pair groups per op

              def conv_band(seg, band):
                  if seg >= NCK:
                      return
                  ns = 512 * seg
                  if band == 0:
                      acc_dve[seg] = adpool.tile([P, CH, 512], BF16, tag="ad",
                                                 name=f"ad{seg}")
                      acc_pool[seg] = appool.tile([P, CH, 512], BF16, tag="ap",
                                                  name=f"ap{seg}")
                  ad, ap_ = acc_dve[seg], acc_pool[seg]
                  for kind, o in CONV_BANDS[band]:
                      lo = max(ns, o)
                      if lo >= ns + 512:
                          continue
                      cs = lo - ns  # start col within the segment
                      w = 512 - cs
                      oi = OFFSETS.index(o)
                      vw = VCP if kind in ("p", "q") else 2 * VCP
                      for q in range(CH // vw):
                          v0, v1 = vw * q, vw * (q + 1)
                          src = field[:, v0:v1, lo - o:ns + 512 - o]
                          if kind == "i":
                              nc.vector.tensor_scalar_mul(
                                  ad[:, v0:v1, cs:512], src, w_rep[:, oi:oi + 1])
                          elif kind == "q":
                              nc.vector.tensor_scalar_mul(
                                  ap_[:, v0:v1, cs:512], src, w_rep[:, oi:oi + 1])
                          elif kind == "a":
                              tmp = tmpapool.tile([P, 2 * VCP, 512], BF16, tag="tmpa")
                              nc.scalar.activation(
                                  out=tmp[:, :, 0:w], in_=src,
                                  func=mybir.ActivationFunctionType.Copy,
                                  scale=w_rep[:, oi:oi + 1],
                              )
                              nc.vector.tensor_add(
                                  ad[:, v0:v1, cs:512], ad[:, v0:v1, cs:512],
                                  tmp[:, :, 0:w])
                          elif kind == "p":
                              tmp = tmpqpool.tile([P, VCP, 512], BF16, tag="tmpq")
                              nc.vector.tensor_scalar_mul(
                                  tmp[:, :, 0:w], src, w_rep[:, oi:oi + 1])
                              if os.environ.get("DBG_NO_POOL"):
                                  nc.vector.tensor_add(
                                      ap_[:, v0:v1, cs:512],
                                      ap_[:, v0:v1, cs:512], tmp[:, :, 0:w])
                              else:
                                  nc.gpsimd.tensor_add(
                                      ap_[:, v0:v1, cs:512],
                                      ap_[:, v0:v1, cs:512], tmp[:, :, 0:w])
                          else:  # "d": DVE scale + DVE add
                              tmp = tmpdpool.tile([P, 2 * VCP, 512], BF16,
                                                  tag="tmpd", name="tmpw")
                              nc.vector.tensor_scalar_mul(
                                  tmp[:, :, 0:w], src, w_rep[:, oi:oi + 1])
                              nc.vector.tensor_add(
                                  ad[:, v0:v1, cs:512], ad[:, v0:v1, cs:512],
                                  tmp[:, :, 0:w])

              def gate_seg(seg, gate):
                  ns = 512 * seg
                  for gc in range(CH):
                      ps = pspool.tile([P, 512], F32, tag="mm")
                      for ic in range(4):
                          nc.tensor.matmul(
                              ps[:, :],
                              lhsT=wg8_sb[:, ic, :, P * gc:P * (gc + 1)],
                              rhs=x8_sb[:, ic, :, ns:ns + 512],
                              perf_mode=mybir.MatmulPerfMode.DoubleRow,
                              start=(ic == 0), stop=(ic == 3),
                          )
                      nc.scalar.activation(
                          out=gate[:, gc, :], in_=ps[:, :],
                          func=mybir.ActivationFunctionType.Sigmoid,
                          bias=bg_sb[:, gc:gc + 1], scale=1.0,
                      )

              # band schedule: conv(s) only needs field column blocks <= s,
              # so bands stream out at pacing points starting inside the v
              # phase.  acc pool rotation throttles how far ahead this runs.
              NB = len(CONV_BANDS)
              sched = [(s, b) for s in range(NCK) for b in range(NB)]
              spos = [0]

              def emit_bands(k):
                  while k > 0 and spos[0] < len(sched):
                      s, b = sched[spos[0]]
                      spos[0] += 1
                      conv_band(s, b)
                      k -= 1


              for _nch in range(NCK):
                  k_phase_chunk(_nch)
              v_chunk(0)
              emit_bands(2)
              v_chunk(1)
              emit_bands(4)
              v_chunk(2)
              emit_bands(4)
              # gate(0) hoisted into the v phase: its sigmoids land early in
              # the Act queue so the first output segment's psum slots free
              # promptly
              gate0 = gpool.tile([P, CH, 512], BF16, tag="gate")
              gate_seg(0, gate0)
              v_chunk(3)
              emit_bands(4)

              # ========= per-segment: gate, coupling + gate-mul, out proj ==
              gated = x_sb  # dead after the v phase; reused in d-major layout

              def couple_seg(seg, gate):
                  ns = 512 * seg
                  ad, ap_ = acc_dve.pop(seg), acc_pool.pop(seg)
                  lo0 = max(0, INIT_OFF - ns)
                  lop = max(0, POOL_INIT - ns)
                  # merge the Pool partial into acc_dve (2x-mode adds) so the
                  # coupling needs one acc pass instead of two
                  for q in range(2):
                      v0, v1 = 4 * q, 4 * (q + 1)
                      nc.vector.tensor_add(
                          ad[:, v0:v1, lop:512], ad[:, v0:v1, lop:512],
                          ap_[:, v0:v1, lop:512])
                  for vc in range(CH):
                      ps = pspool.tile([P, 512], F32, tag="mm")
                      for gi, o in enumerate(COUPLE_CONV):
                          lo = max(0, o - ns)
                          # gi == 0 is offset 0 (full width) -> start pass
                          nc.tensor.matmul(
                              ps[:, lo:512],
                              lhsT=gcoup[:, gi, :],
                              rhs=field[:, vc, ns + lo - o:ns + 512 - o],
                              start=(gi == 0), stop=False,
                          )
                      nc.tensor.matmul(
                          ps[:, lo0:512], lhsT=g_sb[:, :], rhs=ad[:, vc, lo0:512],
                          start=False, stop=True,
                      )
                      nc.vector.tensor_mul(
                          gated[:, vc, ns:ns + 512], ps[:, :], gate[:, vc, :],
                      )

              def out_tile(nt):
                  for fch in range(2):
                      fs = 512 * fch
                      ps = pspool.tile([P, 512], F32, tag="mm")
                      for vc in range(CH):
                          nc.tensor.matmul(
                              ps[:, :],
                              lhsT=gated[:, vc, P * nt:P * (nt + 1)],
                              rhs=wo_sb[:, vc, fs:fs + 512],
                              start=(vc == 0), stop=(vc == CH - 1),
                          )
                      ystg = ypool.tile([P, 512], F32, tag="y")
                      if ob_zero:
                          nc.scalar.activation(
                              out=ystg[:, :], in_=ps[:, :],
                              func=mybir.ActivationFunctionType.Copy,
                          )
                      else:
                          nc.vector.tensor_add(
                              ystg[:, :], ps[:, :], ob_sb[:, fs:fs + 512],
                          )
                      nc.sync.dma_start(out=y_d[P * nt:P * (nt + 1), fs:fs + 512],
                                        in_=ystg[:, :])

              for seg in range(NCK):
                  if seg == 0:
                      gate = gate0
                  else:
                      gate = gpool.tile([P, CH, 512], BF16, tag="gate")
                      gate_seg(seg, gate)
                  couple_seg(seg, gate)
                  emit_bands(2)
                  for nt in range(4 * seg, 4 * seg + 4):
                      out_tile(nt)
                      emit_bands(1)

    nc.compile()
    return nc


def _prep_shared(qkv_w, qkv_b, out_w, out_b, gate_w, gate_b, scale_gain, field_coupling):
    perm = PERM
    wk8 = np.ascontiguousarray(qkv_w[D:2 * D, :].T.astype(NP_FP8))
    wv = np.ascontiguousarray(qkv_w[2 * D:3 * D, :][perm, :].T.astype(NP_BF16))
    wg8 = np.ascontiguousarray(gate_w[perm, :].T.astype(NP_FP8))
    wo = np.ascontiguousarray(out_w[:, perm].T.astype(NP_BF16))
    bk = np.ascontiguousarray(qkv_b[D:2 * D].reshape(CH, P).T.astype(np.float32))
    bv = np.ascontiguousarray(qkv_b[2 * D:3 * D][perm].reshape(CH, P).T.astype(np.float32))
    bg = np.ascontiguousarray(gate_b[perm].reshape(CH, P).T.astype(np.float32))
    ob = np.ascontiguousarray(np.broadcast_to(out_b.astype(np.float32), (P, D)))
    sg = np.ascontiguousarray(np.tile(scale_gain.T.astype(np.float32),
                                      (P // H, 1)))
    fc = np.ascontiguousarray(field_coupling.astype(np.float32))
    return {"wk8": wk8, "wv": wv, "wg8": wg8, "wo": wo, "bk": bk,
            "bv": bv, "bg": bg, "ob": ob, "sg": sg, "fc": fc}


def _make_in_maps(x, shared):
    in_maps = []
    for b in range(B):
        m = dict(shared)
        xt = np.ascontiguousarray(x[b].T)
        m["x_cm"] = xt.astype(NP_BF16)
        m["x8f"] = xt.astype(NP_FP8)
        in_maps.append(m)
    return in_maps


def kernel(x, qkv_w, qkv_b, out_w, out_b, gate_w, gate_b, scale_gain,
           field_coupling):
    x = np.asarray(x, dtype=np.float32)
    qkv_w = np.asarray(qkv_w, dtype=np.float32)
    qkv_b = np.asarray(qkv_b, dtype=np.float32)
    out_w = np.asarray(out_w, dtype=np.float32)
    out_b = np.asarray(out_b, dtype=np.float32)
    gate_w = np.asarray(gate_w, dtype=np.float32)
    gate_b = np.asarray(gate_b, dtype=np.float32)
    scale_gain = np.asarray(scale_gain, dtype=np.float32)
    field_coupling = np.asarray(field_coupling, dtype=np.float32)

    ob_zero = not np.any(out_b)
    key = ("nc", ob_zero)
    if key not in _CACHE:
        _CACHE[key] = _build_program(ob_zero=ob_zero)
    nc = _CACHE[key]

    shared = _prep_shared(qkv_w, qkv_b, out_w, out_b, gate_w, gate_b,
                          scale_gain, field_coupling)
    in_maps = _make_in_maps(x, shared)

    res = bass_utils.run_bass_kernel_spmd(nc, in_maps, list(range(NCORES)))
    out = np.stack([np.asarray(res.results[b]["y"], dtype=np.float32)
                    for b in range(B)], axis=0)
    return out
